# revision 56
# baseline (speedup 1.0000x reference)
"""Trainium2 Bass kernel for nn_PredicateTensorModel.

Math (reference):
  subj/verb/obj[c,d] = weighted embedding bags (N=8 ids per batch row)
  A[c,p,q]  = sum_i w[i,p,q] verb[c,i]
  US[c,p,q] = sum_j u[j,p,q] subj[c,j]
  out[c,q]  = sum_p US[c,p,q] * A[c,p,q] * obj[c,p]

Sharding: tensor-parallel over trailing q axis (32 q's per core, 8 cores).
w/u are pre-transposed on host to [i, q, p] layout and sharded contiguous;
all TensorEngine-facing data is bf16 (f32 PSUM accumulation).

Structure:
  - Embedding rows are fetched with 96 per-chunk indirect DMAs (128 rows
    each -- the HW consumes exactly one offset per partition, so larger
    multi-offset gathers are impossible and the ~1us/instruction SWDGE
    cost on the Pool engine is the kernel's critical path, pipelined
    against everything else by c-quarters).
  - w/u slices live fully resident in SBUF ([128, 8192] bf16 tiles),
    loaded with 16 [128, 1024] DMAs to keep HWDGE overhead low.
  - Per c-quarter, gathers run s, o, v and tiles are split in two passes
    so only the psA matmul + accumulating STT remain after the quarter's
    last (v) gather: pass 1 = psU matmul, Act copy to bf16, G = USs*obj
    on DVE (all possible from the s/o bags); pass 2 = psA matmul + two
    accumulating scalar_tensor_tensor ops that also reduce over p.
"""

import os
import sys

sys.path.insert(0, "/opt/trn_rl_repo")

import numpy as np
import ml_dtypes

N_CORES = 8
VOCAB, D, B, N = 50000, 256, 512, 8
QS = D // N_CORES  # 32 q columns per core
NCHUNK = B // 16  # 32 gather chunks of 16 batch rows
NQUART = 4  # c-quarters (128 batch rows each)
CHQ = NCHUNK // NQUART  # 8 chunks per quarter
CHH = NCHUNK // 2  # 16 chunks per gather half

bf16 = ml_dtypes.bfloat16

_PROG_CACHE = {}

# Fraction control: tiles with (index % 8) < POOL_MOD run the G multiply
# on gpsimd; the rest on DVE.
POOL_MOD = 5


def _build_program():
    import concourse.bass as bass
    import concourse.tile as tile
    import concourse.mybir as mybir
    from concourse import bacc
    from contextlib import ExitStack

    dt = mybir.dt
    nc = bacc.Bacc()

    emb_p = nc.declare_dram_parameter("emb_b", [VOCAB, D], dt.bfloat16, isOutput=False)
    w_p = nc.declare_dram_parameter("w_k", [D, QS, D], dt.bfloat16, isOutput=False)
    u_p = nc.declare_dram_parameter("u_k", [D, QS, D], dt.bfloat16, isOutput=False)
    ids_p = {}
    S_p = {}
    for t in "svo":
        ids_p[t] = nc.declare_dram_parameter(f"ids_{t}", [128, NCHUNK], dt.int32, isOutput=False)
        S_p[t] = nc.declare_dram_parameter(f"S_{t}", [128, B], dt.bfloat16, isOutput=False)
    ident_p = nc.declare_dram_parameter("ident", [128, 128], dt.bfloat16, isOutput=False)
    out_p = nc.declare_dram_parameter("out", [B, QS], dt.float32, isOutput=True)

    with ExitStack() as ctx:
        tc = ctx.enter_context(tile.TileContext(nc))
        const_pool = ctx.enter_context(tc.tile_pool(name="const", bufs=1))
        gather_pool = ctx.enter_context(tc.tile_pool(name="gather", bufs=96))
        embT_pool = ctx.enter_context(tc.tile_pool(name="embT", bufs=1))
        wu_pool = ctx.enter_context(tc.tile_pool(name="wu", bufs=1))
        stage_pool = ctx.enter_context(tc.tile_pool(name="stage", bufs=6))
        g_pool = ctx.enter_context(tc.tile_pool(name="gstage", bufs=16))
        us_pool = ctx.enter_context(tc.tile_pool(name="usstage", bufs=16))
        out_pool = ctx.enter_context(tc.tile_pool(name="outp", bufs=1))
        psum_pool = ctx.enter_context(tc.tile_pool(name="ps", bufs=7, space="PSUM"))
        psum_small = ctx.enter_context(tc.tile_pool(name="pssm", bufs=1, space="PSUM"))

        # ---- constants ----
        idt = {}
        St = {}
        for t in "svo":
            idt[t] = const_pool.tile([128, NCHUNK], dt.int32, name=f"ids{t}", tag=f"ids{t}")
            nc.sync.dma_start(out=idt[t][:], in_=ids_p[t][:])
            St[t] = const_pool.tile([128, B], dt.bfloat16, name=f"S{t}", tag=f"S{t}")
            nc.sync.dma_start(out=St[t][:], in_=S_p[t][:])
        ident = const_pool.tile([128, 128], dt.bfloat16, name="ident", tag="ident")
        nc.sync.dma_start(out=ident[:], in_=ident_p[:])
        dummy = const_pool.tile([1, 1], dt.int32, name="dummy", tag="dummy")
        # Warm the gpsimd engine clock on all three ids loads up front, so no
        # later indirect DMA needs a second sync wait for them.
        for t in "svo":
            nc.gpsimd.tensor_copy(dummy[:], idt[t][:1, :1])

        # ---- resident w/u tiles ----
        # w_res[ic] holds w[ic*128:(ic+1)*128, :, :] as [128, 32q x 256p].
        w_res = []
        u_res = []
        for ic in range(2):
            w_res.append(wu_pool.tile([128, QS * D], dt.bfloat16, name=f"w{ic}", tag=f"w{ic}"))
            u_res.append(wu_pool.tile([128, QS * D], dt.bfloat16, name=f"u{ic}", tag=f"u{ic}"))

        def load_wu_quad(o4):
            # 4 q columns = 1024 elements for each of w0/w1/u0/u1
            for ic in range(2):
                nc.sync.dma_start(
                    out=w_res[ic][:, o4 * 1024 : (o4 + 1) * 1024],
                    in_=w_p[ic * 128 : (ic + 1) * 128, o4 * 4 : (o4 + 1) * 4, :],
                )
                nc.sync.dma_start(
                    out=u_res[ic][:, o4 * 1024 : (o4 + 1) * 1024],
                    in_=u_p[ic * 128 : (ic + 1) * 128, o4 * 4 : (o4 + 1) * 4, :],
                )

        # ---- phase E state ----
        embT = {
            t: [
                embT_pool.tile([128, B], dt.bfloat16, name=f"eT{t}{dh}", tag=f"eT{t}{dh}")
                for dh in range(2)
            ]
            for t in "sv"
        }
        embT_o = [
            embT_pool.tile([128, B], dt.bfloat16, name=f"eTo{dh}", tag=f"eTo{dh}")
            for dh in range(2)
        ]
        # obj stored doubled: [c, p] block twice along free axis, so the
        # G multiply is a single [128, 512] tensor_tensor per tile.
        obj_s = [
            embT_pool.tile([128, 2 * D], dt.bfloat16, name=f"objs{k}", tag=f"obj{k}")
            for k in range(NQUART)
        ]

        def gather_chunk(t, ck):
            # One offset per partition is all the HW indirect DMA supports:
            # each instruction gathers exactly 128 rows (one 16-batch chunk).
            V = gather_pool.tile([128, D], dt.bfloat16, name=f"V{t}{ck}", tag="V")
            nc.gpsimd.indirect_dma_start(
                out=V[:],
                out_offset=None,
                in_=emb_p[:],
                in_offset=bass.IndirectOffsetOnAxis(
                    ap=idt[t][:, ck : ck + 1], axis=0
                ),
            )
            return V

        def bags_one(k, Vs, t):
            # [128, 256] psum: (t, dh0) in cols 0:128, (t, dh1) in 128:256
            psE = psum_pool.tile([128, 256], dt.float32, name=f"psE{t}", tag="ps")
            for dh in range(2):
                for c8 in range(CHQ):
                    nc.tensor.matmul(
                        out=psE[:, dh * 128 + c8 * 16 : dh * 128 + (c8 + 1) * 16],
                        lhsT=Vs[t][c8][:, dh * 128 : (dh + 1) * 128],
                        rhs=St[t][:, (k * CHQ + c8) * 16 : (k * CHQ + c8 + 1) * 16],
                        start=True,
                        stop=True,
                    )
            cb = k * 128
            dest = embT_o if t == "o" else embT[t]
            for dh in range(2):
                # the v copies feed the tail-critical psA matmuls; Act is
                # busy with USs copies then, DVE is idle
                if t == "v":
                    nc.vector.tensor_copy(dest[dh][:, cb : cb + 128], psE[:, dh * 128 : (dh + 1) * 128])
                else:
                    nc.scalar.copy(out=dest[dh][:, cb : cb + 128], in_=psE[:, dh * 128 : (dh + 1) * 128])
            if t == "o":
                # transpose obj back to [c, p] for this c-quarter; store twice
                for dh in range(2):
                    ptr = psum_small.tile([128, 128], dt.bfloat16, name="ptr", tag="pssm")
                    nc.tensor.transpose(
                        out=ptr[:],
                        in_=embT_o[dh][:, cb : cb + 128],
                        identity=ident[:],
                    )
                    nc.scalar.copy(out=obj_s[k][:, dh * 128 : (dh + 1) * 128], in_=ptr[:])
                    nc.scalar.copy(out=obj_s[k][:, D + dh * 128 : D + (dh + 1) * 128], in_=ptr[:])

        outs = [
            out_pool.tile([128, QS], dt.float32, name=f"outs{k}", tag=f"out{k}")
            for k in range(NQUART)
        ]

        tile_idx = 0
        USs_t = {}
        G_t = {}

        def tile_u_pass(k, qj):
            # psU matmuls + USs copy: needs only the subj bags
            psU = psum_pool.tile([128, 512], dt.float32, name="psU", tag="ps")
            cb = k * 128
            for ic in range(2):
                nc.tensor.matmul(
                    out=psU[:],
                    lhsT=embT["s"][ic][:, cb : cb + 128],
                    rhs=u_res[ic][:, qj * 512 : (qj + 1) * 512],
                    start=(ic == 0),
                    stop=(ic == 1),
                )
            USs = stage_pool.tile([128, 512], dt.bfloat16, name="USs", tag="USs")
            nc.scalar.copy(out=USs[:], in_=psU[:])
            G = g_pool.tile([128, 512], dt.bfloat16, name="G", tag="G")
            nc.vector.tensor_mul(G[:], USs[:], obj_s[k][:])
            G_t[qj] = G

        def tile_a_pass(k, qj):
            # psA matmuls + accumulating STT: the only work gated on the
            # quarter's last gather (v)
            psA = psum_pool.tile([128, 512], dt.float32, name="psA", tag="ps")
            cb = k * 128
            for ic in range(2):
                nc.tensor.matmul(
                    out=psA[:],
                    lhsT=embT["v"][ic][:, cb : cb + 128],
                    rhs=w_res[ic][:, qj * 512 : (qj + 1) * 512],
                    start=(ic == 0),
                    stop=(ic == 1),
                )
            G = G_t.pop(qj)
            junk = stage_pool.tile([128, D], dt.bfloat16, name="junk", tag="junk")
            for qq in range(2):
                q_col = qj * 2 + qq
                nc.vector.scalar_tensor_tensor(
                    out=junk[:],
                    in0=psA[:, qq * D : (qq + 1) * D],
                    scalar=1.0,
                    in1=G[:, qq * D : (qq + 1) * D],
                    op0=mybir.AluOpType.mult,
                    op1=mybir.AluOpType.mult,
                    accum_out=outs[k][:, q_col : q_col + 1],
                )

        # Gathers trickle out of the Pool engine at ~1us each (SWDGE fixed
        # cost); they are the critical path. wu loads are cheap on the DMA
        # device and can all go early. Quarter k's gathers are ordered
        # s, v (matmul inputs) then o (needed a bit later for the STT).
        Vq = {}
        for k in range(NQUART):
            Vq[k] = {
                t: [gather_chunk(t, k * CHQ + c8) for c8 in range(CHQ)]
                for t in "sov"
            }
            load_wu_quad(2 * k)
            load_wu_quad(2 * k + 1)

        # quarter-major: each c-quarter's 16 tiles run while the next
        # quarter's gathers stream on the Pool engine. obj bags are emitted
        # after the quarter's tiles: the PE stream then runs the tiles'
        # matmuls (which need only v/s) without stalling on the o-gathers.
        for k in range(NQUART):
            bags_one(k, Vq[k], "s")
            bags_one(k, Vq[k], "o")
            for qj in range(16):
                tile_u_pass(k, qj)
            bags_one(k, Vq[k], "v")
            for qj in range(16):
                tile_a_pass(k, qj)
            nc.sync.dma_start(
                out=out_p[k * 128 : (k + 1) * 128, :], in_=outs[k][:]
            )

    nc.finalize()
    return nc


def _get_program():
    if "nc" not in _PROG_CACHE:
        _PROG_CACHE["nc"] = _build_program()
    return _PROG_CACHE["nc"]


def _host_prep(inputs):
    """Shard + lay out inputs for the 8 cores. Returns list of in_maps."""
    ids = {}
    wts = {}
    for t, idk, wk in (
        ("s", "subj_id", "subj_w"),
        ("v", "verb_id", "verb_w"),
        ("o", "obj_id", "obj_w"),
    ):
        ids[t] = np.asarray(inputs[idk]).astype(np.int32)
        wts[t] = np.asarray(inputs[wk]).astype(np.float32)

    emb = np.asarray(inputs["emb"], dtype=np.float32)
    w = np.asarray(inputs["w"], dtype=np.float32)
    u = np.asarray(inputs["u"], dtype=np.float32)

    emb_b = emb.astype(bf16)
    # [i, p, q] -> [i, q, p], contiguous, then shard q
    wT = np.ascontiguousarray(w.transpose(0, 2, 1)).astype(bf16)
    uT = np.ascontiguousarray(u.transpose(0, 2, 1)).astype(bf16)

    ids_r = {}
    S_m = {}
    for t in "svo":
        # partition p = (c % 16)*8 + n ; column = chunk ck = c // 16
        ids_r[t] = np.ascontiguousarray(
            ids[t].reshape(NCHUNK, 16, 8).transpose(1, 2, 0).reshape(128, NCHUNK)
        )
        Sm = np.zeros((16, 8, NCHUNK, 16), np.float32)
        wr = wts[t].reshape(NCHUNK, 16, 8).transpose(1, 2, 0)  # [16 j, 8 n, 32 ck]
        j = np.arange(16)
        Sm[j[:, None, None], np.arange(8)[None, :, None], np.arange(NCHUNK)[None, None, :], j[:, None, None]] = wr
        S_m[t] = np.ascontiguousarray(Sm.reshape(128, B)).astype(bf16)

    ident = np.eye(128, dtype=bf16)

    in_maps = []
    for k in range(N_CORES):
        m = {
            "emb_b": emb_b,
            "w_k": np.ascontiguousarray(wT[:, k * QS : (k + 1) * QS, :]),
            "u_k": np.ascontiguousarray(uT[:, k * QS : (k + 1) * QS, :]),
            "ident": ident,
        }
        for t in "svo":
            m[f"ids_{t}"] = ids_r[t]
            m[f"S_{t}"] = S_m[t]
        in_maps.append(m)
    return in_maps


def kernel(**inputs) -> np.ndarray:
    from concourse.bass_utils import run_bass_kernel_spmd

    nc = _get_program()
    in_maps = _host_prep(inputs)
    trace = bool(int(os.environ.get("KTRACE", "0")))
    res = run_bass_kernel_spmd(
        nc, in_maps, core_ids=list(range(N_CORES)), trace=trace
    )
    if trace:
        _PROG_CACHE["last_result"] = res
    out = np.concatenate(
        [res.results[k]["out"].astype(np.float32) for k in range(N_CORES)], axis=1
    )
    return out


# revision 57
# speedup vs baseline: 1.0023x; 1.0023x over previous
"""Trainium2 Bass kernel for nn_PredicateTensorModel.

Math (reference):
  subj/verb/obj[c,d] = weighted embedding bags (N=8 ids per batch row)
  A[c,p,q]  = sum_i w[i,p,q] verb[c,i]
  US[c,p,q] = sum_j u[j,p,q] subj[c,j]
  out[c,q]  = sum_p US[c,p,q] * A[c,p,q] * obj[c,p]

Sharding: tensor-parallel over trailing q axis (32 q's per core, 8 cores).
w/u are pre-transposed on host to [i, q, p] layout and sharded contiguous;
all TensorEngine-facing data is bf16 (f32 PSUM accumulation).

Structure:
  - Embedding rows are fetched with 96 per-chunk indirect DMAs (128 rows
    each -- the HW consumes exactly one offset per partition, so larger
    multi-offset gathers are impossible and the ~1us/instruction SWDGE
    cost on the Pool engine is the kernel's critical path, pipelined
    against everything else by c-quarters).
  - w/u slices live fully resident in SBUF ([128, 8192] bf16 tiles),
    loaded with 16 [128, 1024] DMAs to keep HWDGE overhead low.
  - Per c-quarter, gathers run s, o, v and tiles are split in two passes
    so only the psA matmul + accumulating STT remain after the quarter's
    last (v) gather: pass 1 = psU matmul, Act copy to bf16, G = USs*obj
    on DVE (all possible from the s/o bags); pass 2 = psA matmul + two
    accumulating scalar_tensor_tensor ops that also reduce over p.
"""

import os
import sys

sys.path.insert(0, "/opt/trn_rl_repo")

import numpy as np
import ml_dtypes

N_CORES = 8
VOCAB, D, B, N = 50000, 256, 512, 8
QS = D // N_CORES  # 32 q columns per core
NCHUNK = B // 16  # 32 gather chunks of 16 batch rows
NQUART = 4  # c-quarters (128 batch rows each)
CHQ = NCHUNK // NQUART  # 8 chunks per quarter
CHH = NCHUNK // 2  # 16 chunks per gather half

bf16 = ml_dtypes.bfloat16

_PROG_CACHE = {}

# Fraction control: tiles with (index % 8) < POOL_MOD run the G multiply
# on gpsimd; the rest on DVE.
POOL_MOD = 5


def _build_program():
    import concourse.bass as bass
    import concourse.tile as tile
    import concourse.mybir as mybir
    from concourse import bacc
    from contextlib import ExitStack

    dt = mybir.dt
    nc = bacc.Bacc()

    emb_p = nc.declare_dram_parameter("emb_b", [VOCAB, D], dt.bfloat16, isOutput=False)
    w_p = nc.declare_dram_parameter("w_k", [D, QS, D], dt.bfloat16, isOutput=False)
    u_p = nc.declare_dram_parameter("u_k", [D, QS, D], dt.bfloat16, isOutput=False)
    ids_p = {}
    S_p = {}
    for t in "svo":
        ids_p[t] = nc.declare_dram_parameter(f"ids_{t}", [128, NCHUNK], dt.int32, isOutput=False)
        S_p[t] = nc.declare_dram_parameter(f"S_{t}", [128, B], dt.bfloat16, isOutput=False)
    ident_p = nc.declare_dram_parameter("ident", [128, 128], dt.bfloat16, isOutput=False)
    out_p = nc.declare_dram_parameter("out", [B, QS], dt.float32, isOutput=True)

    with ExitStack() as ctx:
        tc = ctx.enter_context(tile.TileContext(nc))
        const_pool = ctx.enter_context(tc.tile_pool(name="const", bufs=1))
        gather_pool = ctx.enter_context(tc.tile_pool(name="gather", bufs=96))
        embT_pool = ctx.enter_context(tc.tile_pool(name="embT", bufs=1))
        wu_pool = ctx.enter_context(tc.tile_pool(name="wu", bufs=1))
        stage_pool = ctx.enter_context(tc.tile_pool(name="stage", bufs=6))
        g_pool = ctx.enter_context(tc.tile_pool(name="gstage", bufs=16))
        us_pool = ctx.enter_context(tc.tile_pool(name="usstage", bufs=16))
        out_pool = ctx.enter_context(tc.tile_pool(name="outp", bufs=1))
        psum_pool = ctx.enter_context(tc.tile_pool(name="ps", bufs=7, space="PSUM"))
        psum_small = ctx.enter_context(tc.tile_pool(name="pssm", bufs=1, space="PSUM"))

        # ---- constants ----
        idt = {}
        St = {}
        for t in "svo":
            idt[t] = const_pool.tile([128, NCHUNK], dt.int32, name=f"ids{t}", tag=f"ids{t}")
            nc.sync.dma_start(out=idt[t][:], in_=ids_p[t][:])
            St[t] = const_pool.tile([128, B], dt.bfloat16, name=f"S{t}", tag=f"S{t}")
            nc.sync.dma_start(out=St[t][:], in_=S_p[t][:])
        ident = const_pool.tile([128, 128], dt.bfloat16, name="ident", tag="ident")
        nc.sync.dma_start(out=ident[:], in_=ident_p[:])

        # ---- resident w/u tiles ----
        # w_res[ic] holds w[ic*128:(ic+1)*128, :, :] as [128, 32q x 256p].
        w_res = []
        u_res = []
        for ic in range(2):
            w_res.append(wu_pool.tile([128, QS * D], dt.bfloat16, name=f"w{ic}", tag=f"w{ic}"))
            u_res.append(wu_pool.tile([128, QS * D], dt.bfloat16, name=f"u{ic}", tag=f"u{ic}"))

        def load_wu_quad(o4):
            # 4 q columns = 1024 elements for each of w0/w1/u0/u1
            for ic in range(2):
                nc.sync.dma_start(
                    out=w_res[ic][:, o4 * 1024 : (o4 + 1) * 1024],
                    in_=w_p[ic * 128 : (ic + 1) * 128, o4 * 4 : (o4 + 1) * 4, :],
                )
                nc.sync.dma_start(
                    out=u_res[ic][:, o4 * 1024 : (o4 + 1) * 1024],
                    in_=u_p[ic * 128 : (ic + 1) * 128, o4 * 4 : (o4 + 1) * 4, :],
                )

        # ---- phase E state ----
        embT = {
            t: [
                embT_pool.tile([128, B], dt.bfloat16, name=f"eT{t}{dh}", tag=f"eT{t}{dh}")
                for dh in range(2)
            ]
            for t in "sv"
        }
        embT_o = [
            embT_pool.tile([128, B], dt.bfloat16, name=f"eTo{dh}", tag=f"eTo{dh}")
            for dh in range(2)
        ]
        # obj stored doubled: [c, p] block twice along free axis, so the
        # G multiply is a single [128, 512] tensor_tensor per tile.
        obj_s = [
            embT_pool.tile([128, 2 * D], dt.bfloat16, name=f"objs{k}", tag=f"obj{k}")
            for k in range(NQUART)
        ]

        def gather_chunk(t, ck):
            # One offset per partition is all the HW indirect DMA supports:
            # each instruction gathers exactly 128 rows (one 16-batch chunk).
            V = gather_pool.tile([128, D], dt.bfloat16, name=f"V{t}{ck}", tag="V")
            nc.gpsimd.indirect_dma_start(
                out=V[:],
                out_offset=None,
                in_=emb_p[:],
                in_offset=bass.IndirectOffsetOnAxis(
                    ap=idt[t][:, ck : ck + 1], axis=0
                ),
            )
            return V

        def bags_one(k, Vs, t):
            # [128, 256] psum: (t, dh0) in cols 0:128, (t, dh1) in 128:256
            psE = psum_pool.tile([128, 256], dt.float32, name=f"psE{t}", tag="ps")
            for dh in range(2):
                for c8 in range(CHQ):
                    nc.tensor.matmul(
                        out=psE[:, dh * 128 + c8 * 16 : dh * 128 + (c8 + 1) * 16],
                        lhsT=Vs[t][c8][:, dh * 128 : (dh + 1) * 128],
                        rhs=St[t][:, (k * CHQ + c8) * 16 : (k * CHQ + c8 + 1) * 16],
                        start=True,
                        stop=True,
                    )
            cb = k * 128
            dest = embT_o if t == "o" else embT[t]
            for dh in range(2):
                # the v copies feed the tail-critical psA matmuls; Act is
                # busy with USs copies then, DVE is idle
                if t == "v":
                    nc.vector.tensor_copy(dest[dh][:, cb : cb + 128], psE[:, dh * 128 : (dh + 1) * 128])
                else:
                    nc.scalar.copy(out=dest[dh][:, cb : cb + 128], in_=psE[:, dh * 128 : (dh + 1) * 128])
            if t == "o":
                # transpose obj back to [c, p] for this c-quarter; store twice
                for dh in range(2):
                    ptr = psum_small.tile([128, 128], dt.bfloat16, name="ptr", tag="pssm")
                    nc.tensor.transpose(
                        out=ptr[:],
                        in_=embT_o[dh][:, cb : cb + 128],
                        identity=ident[:],
                    )
                    nc.scalar.copy(out=obj_s[k][:, dh * 128 : (dh + 1) * 128], in_=ptr[:])
                    nc.scalar.copy(out=obj_s[k][:, D + dh * 128 : D + (dh + 1) * 128], in_=ptr[:])

        outs = [
            out_pool.tile([128, QS], dt.float32, name=f"outs{k}", tag=f"out{k}")
            for k in range(NQUART)
        ]

        tile_idx = 0
        USs_t = {}
        G_t = {}

        def tile_u_pass(k, qj):
            # psU matmuls + USs copy: needs only the subj bags
            psU = psum_pool.tile([128, 512], dt.float32, name="psU", tag="ps")
            cb = k * 128
            for ic in range(2):
                nc.tensor.matmul(
                    out=psU[:],
                    lhsT=embT["s"][ic][:, cb : cb + 128],
                    rhs=u_res[ic][:, qj * 512 : (qj + 1) * 512],
                    start=(ic == 0),
                    stop=(ic == 1),
                )
            USs = stage_pool.tile([128, 512], dt.bfloat16, name="USs", tag="USs")
            nc.scalar.copy(out=USs[:], in_=psU[:])
            G = g_pool.tile([128, 512], dt.bfloat16, name="G", tag="G")
            nc.vector.tensor_mul(G[:], USs[:], obj_s[k][:])
            G_t[qj] = G

        def tile_a_pass(k, qj):
            # psA matmuls + accumulating STT: the only work gated on the
            # quarter's last gather (v)
            psA = psum_pool.tile([128, 512], dt.float32, name="psA", tag="ps")
            cb = k * 128
            for ic in range(2):
                nc.tensor.matmul(
                    out=psA[:],
                    lhsT=embT["v"][ic][:, cb : cb + 128],
                    rhs=w_res[ic][:, qj * 512 : (qj + 1) * 512],
                    start=(ic == 0),
                    stop=(ic == 1),
                )
            G = G_t.pop(qj)
            junk = stage_pool.tile([128, D], dt.bfloat16, name="junk", tag="junk")
            for qq in range(2):
                q_col = qj * 2 + qq
                nc.vector.scalar_tensor_tensor(
                    out=junk[:],
                    in0=psA[:, qq * D : (qq + 1) * D],
                    scalar=1.0,
                    in1=G[:, qq * D : (qq + 1) * D],
                    op0=mybir.AluOpType.mult,
                    op1=mybir.AluOpType.mult,
                    accum_out=outs[k][:, q_col : q_col + 1],
                )

        # Gathers trickle out of the Pool engine at ~1us each (SWDGE fixed
        # cost); they are the critical path. wu loads are cheap on the DMA
        # device and can all go early. Quarter k's gathers are ordered
        # s, v (matmul inputs) then o (needed a bit later for the STT).
        Vq = {}
        for k in range(NQUART):
            Vq[k] = {
                t: [gather_chunk(t, k * CHQ + c8) for c8 in range(CHQ)]
                for t in "sov"
            }
            load_wu_quad(2 * k)
            load_wu_quad(2 * k + 1)

        # quarter-major: each c-quarter's 16 tiles run while the next
        # quarter's gathers stream on the Pool engine. obj bags are emitted
        # after the quarter's tiles: the PE stream then runs the tiles'
        # matmuls (which need only v/s) without stalling on the o-gathers.
        for k in range(NQUART):
            bags_one(k, Vq[k], "s")
            bags_one(k, Vq[k], "o")
            for qj in range(16):
                tile_u_pass(k, qj)
            bags_one(k, Vq[k], "v")
            for qj in range(16):
                tile_a_pass(k, qj)
            nc.sync.dma_start(
                out=out_p[k * 128 : (k + 1) * 128, :], in_=outs[k][:]
            )

    nc.finalize()
    return nc


def _get_program():
    if "nc" not in _PROG_CACHE:
        _PROG_CACHE["nc"] = _build_program()
    return _PROG_CACHE["nc"]


def _host_prep(inputs):
    """Shard + lay out inputs for the 8 cores. Returns list of in_maps."""
    ids = {}
    wts = {}
    for t, idk, wk in (
        ("s", "subj_id", "subj_w"),
        ("v", "verb_id", "verb_w"),
        ("o", "obj_id", "obj_w"),
    ):
        ids[t] = np.asarray(inputs[idk]).astype(np.int32)
        wts[t] = np.asarray(inputs[wk]).astype(np.float32)

    emb = np.asarray(inputs["emb"], dtype=np.float32)
    w = np.asarray(inputs["w"], dtype=np.float32)
    u = np.asarray(inputs["u"], dtype=np.float32)

    emb_b = emb.astype(bf16)
    # [i, p, q] -> [i, q, p], contiguous, then shard q
    wT = np.ascontiguousarray(w.transpose(0, 2, 1)).astype(bf16)
    uT = np.ascontiguousarray(u.transpose(0, 2, 1)).astype(bf16)

    ids_r = {}
    S_m = {}
    for t in "svo":
        # partition p = (c % 16)*8 + n ; column = chunk ck = c // 16
        ids_r[t] = np.ascontiguousarray(
            ids[t].reshape(NCHUNK, 16, 8).transpose(1, 2, 0).reshape(128, NCHUNK)
        )
        Sm = np.zeros((16, 8, NCHUNK, 16), np.float32)
        wr = wts[t].reshape(NCHUNK, 16, 8).transpose(1, 2, 0)  # [16 j, 8 n, 32 ck]
        j = np.arange(16)
        Sm[j[:, None, None], np.arange(8)[None, :, None], np.arange(NCHUNK)[None, None, :], j[:, None, None]] = wr
        S_m[t] = np.ascontiguousarray(Sm.reshape(128, B)).astype(bf16)

    ident = np.eye(128, dtype=bf16)

    in_maps = []
    for k in range(N_CORES):
        m = {
            "emb_b": emb_b,
            "w_k": np.ascontiguousarray(wT[:, k * QS : (k + 1) * QS, :]),
            "u_k": np.ascontiguousarray(uT[:, k * QS : (k + 1) * QS, :]),
            "ident": ident,
        }
        for t in "svo":
            m[f"ids_{t}"] = ids_r[t]
            m[f"S_{t}"] = S_m[t]
        in_maps.append(m)
    return in_maps


def kernel(**inputs) -> np.ndarray:
    from concourse.bass_utils import run_bass_kernel_spmd

    nc = _get_program()
    in_maps = _host_prep(inputs)
    trace = bool(int(os.environ.get("KTRACE", "0")))
    res = run_bass_kernel_spmd(
        nc, in_maps, core_ids=list(range(N_CORES)), trace=trace
    )
    if trace:
        _PROG_CACHE["last_result"] = res
    out = np.concatenate(
        [res.results[k]["out"].astype(np.float32) for k in range(N_CORES)], axis=1
    )
    return out
